# revision 6
# baseline (speedup 1.0000x reference)
"""GCN link-prediction kernel for 8 Trainium2 NeuronCores.

Strategy:
  - Nodes (dst) sharded across 8 cores (12500 each); each core processes the
    edges whose dst lands in its shard (plus its self-loops).
  - GCN sym-norm factorizes per node: out[d] = dinv[d] * sum_{s in N(d)+d}
    dinv[s]*h[s].  So the per-edge norm disappears: pre-scale rows by dinv
    when writing the dense-matmul result, post-scale the aggregation by dinv.
  - Aggregation: edges sorted by dst block (128 nodes); per 128-edge chunk:
    indirect-DMA gather of h~[src] rows -> one-hot selection matrix built
    on-device from dst_local via iota/is_equal -> PE matmul accumulated in
    PSUM per dst block.
  - One compiled program serves both GCN layers (W2 zero-padded to 128 cols;
    relu vs identity via per-partition threshold input: max(v, thr)).
  - Decode runs as a second program: gather z[i], z[j] per 128-pair chunk,
    multiply + row-reduce on DVE.
Host does index-only prep (degree, sorting, padding) and inter-program
concatenation of shards.
"""
import numpy as np

import concourse.bass as bass
import concourse.bacc as bacc
import concourse.mybir as mybir
import concourse.tile as tile
from concourse.bass_utils import run_bass_kernel_spmd
from concourse.masks import make_identity

f32 = mybir.dt.float32
i32 = mybir.dt.int32

N = 100000
E = 1600000
EL = 1048576
IN = 128
HID = 128
OUT = 64
NCORES = 8
NPC = N // NCORES          # 12500 nodes per core
NBLK = (NPC + 127) // 128  # 98 dst blocks per core
P = 128

_prog_cache = {}


def _prep(edge_index):
    src = np.asarray(edge_index[0], dtype=np.int64)
    dst = np.asarray(edge_index[1], dtype=np.int64)
    deg = np.bincount(dst, minlength=N).astype(np.float64) + 1.0
    dinv = (1.0 / np.sqrt(deg)).astype(np.float32)

    # per (core, block) edge lists, self-loops included
    core_of = dst // NPC
    per_core = []
    counts = np.zeros((NCORES, NBLK), dtype=np.int64)
    for c in range(NCORES):
        m = core_of == c
        s_c = src[m]
        d_c = dst[m] - c * NPC
        loop = np.arange(NPC, dtype=np.int64)
        s_c = np.concatenate([s_c, loop + c * NPC])
        d_c = np.concatenate([d_c, loop])
        blk = d_c // 128
        order = np.argsort(blk, kind="stable")
        s_c, d_c, blk = s_c[order], d_c[order], blk[order]
        per_core.append((s_c, d_c, blk))
        counts[c] = np.bincount(blk, minlength=NBLK)

    kb = ((counts.max(axis=0) + 127) // 128).astype(np.int64)  # chunks per blk
    ktot = int(kb.sum())
    col_off = np.concatenate([[0], np.cumsum(kb)[:-1]])

    srcs = np.zeros((NCORES, 128, ktot), dtype=np.int32)
    dstl = np.full((NCORES, 128, ktot), 999.0, dtype=np.float32)
    for c in range(NCORES):
        s_c, d_c, blk = per_core[c]
        pos = 0
        for b in range(NBLK):
            m = int(counts[c, b])
            cols = int(kb[b])
            spad = np.zeros(cols * 128, dtype=np.int32)
            dpad = np.full(cols * 128, 999.0, dtype=np.float32)
            spad[:m] = s_c[pos:pos + m]
            dpad[:m] = (d_c[pos:pos + m] % 128).astype(np.float32)
            srcs[c, :, col_off[b]:col_off[b] + cols] = spad.reshape(cols, 128).T
            dstl[c, :, col_off[b]:col_off[b] + cols] = dpad.reshape(cols, 128).T
            pos += m

    # dinv arranged per dst block [128, NBLK] and per row tile [128, NT]
    NT = (N + 127) // 128
    dinv_dst = np.ones((NCORES, 128, NBLK), dtype=np.float32)
    for c in range(NCORES):
        v = dinv[c * NPC:(c + 1) * NPC]
        vp = np.ones(NBLK * 128, dtype=np.float32)
        vp[:NPC] = v
        dinv_dst[c] = vp.reshape(NBLK, 128).T
    dr = np.ones(NT * 128, dtype=np.float32)
    dr[:N] = dinv
    dinv_rows = dr.reshape(NT, 128).T.copy()

    return dict(srcs=srcs, dstl=dstl, kb=kb, col_off=col_off,
                dinv_dst=dinv_dst, dinv_rows=dinv_rows, ktot=ktot)


def _build_layer(kb, col_off):
    """One GCN layer: tbl [N,128] -> shard out [NPC,128]."""
    NT = (N + 127) // 128
    nc = bacc.Bacc("TRN2", target_bir_lowering=False, debug=False,
                   num_devices=NCORES)
    tbl = nc.dram_tensor("tbl", [N, P], f32, kind="ExternalInput").ap()
    W = nc.dram_tensor("W", [P, P], f32, kind="ExternalInput").ap()
    brep = nc.dram_tensor("brep", [P, P], f32, kind="ExternalInput").ap()
    thr = nc.dram_tensor("thr", [P, 1], f32, kind="ExternalInput").ap()
    iota = nc.dram_tensor("iota", [P, P], f32, kind="ExternalInput").ap()
    ktot = int(kb.sum())
    srcs = nc.dram_tensor("srcs", [P, ktot], i32, kind="ExternalInput").ap()
    dstl = nc.dram_tensor("dstl", [P, ktot], f32, kind="ExternalInput").ap()
    dinv_dst = nc.dram_tensor("dinv_dst", [P, NBLK], f32,
                              kind="ExternalInput").ap()
    dinv_rows = nc.dram_tensor("dinv_rows", [P, NT], f32,
                               kind="ExternalInput").ap()
    out = nc.dram_tensor("out", [NPC, P], f32, kind="ExternalOutput").ap()

    with tile.TileContext(nc) as tc:
        with (tc.tile_pool(name="const", bufs=1) as cpool,
              tc.tile_pool(name="xin", bufs=8) as xpool,
              tc.tile_pool(name="hs", bufs=8) as hpool,
              tc.tile_pool(name="g", bufs=24) as gpool,
              tc.tile_pool(name="m", bufs=8) as mpool,
              tc.tile_pool(name="ob", bufs=8) as opool,
              tc.tile_pool(name="psA", bufs=2, space="PSUM") as psA,
              tc.tile_pool(name="psB", bufs=4, space="PSUM") as psB,
              tc.tile_pool(name="dram", bufs=1, space="DRAM") as dpool):
            htab = dpool.tile([N, P], f32, name="htab")
            W_t = cpool.tile([P, P], f32, name="W_t")
            nc.sync.dma_start(out=W_t[:], in_=W[:])
            brep_t = cpool.tile([P, P], f32, name="brep_t")
            nc.sync.dma_start(out=brep_t[:], in_=brep[:])
            thr_t = cpool.tile([P, 1], f32, name="thr_t")
            nc.sync.dma_start(out=thr_t[:], in_=thr[:])
            iota_t = cpool.tile([P, P], f32, name="iota_t")
            nc.sync.dma_start(out=iota_t[:], in_=iota[:])
            srcs_t = cpool.tile([P, ktot], i32, name="srcs_t")
            nc.sync.dma_start(out=srcs_t[:], in_=srcs[:])
            dstl_t = cpool.tile([P, ktot], f32, name="dstl_t")
            nc.sync.dma_start(out=dstl_t[:], in_=dstl[:])
            dd_t = cpool.tile([P, NBLK], f32, name="dd_t")
            nc.sync.dma_start(out=dd_t[:], in_=dinv_dst[:])
            dr_t = cpool.tile([P, NT], f32, name="dr_t")
            nc.sync.dma_start(out=dr_t[:], in_=dinv_rows[:])
            ident = cpool.tile([P, P], f32, name="ident")
            make_identity(nc, ident[:])

            # dense: htab = dinv * (tbl @ W)
            for t in range(NT):
                r0 = t * 128
                rows = min(128, N - r0)
                xt_ = xpool.tile([P, P], f32, name="xt", tag="xt")
                nc.sync.dma_start(out=xt_[:rows, :], in_=tbl[r0:r0+rows, :])
                psT = psA.tile([P, P], f32, name="psT", tag="psT")
                nc.tensor.transpose(out=psT[:, :rows], in_=xt_[:rows, :],
                                    identity=ident[:rows, :rows])
                xT = xpool.tile([P, P], f32, name="xT", tag="xT")
                nc.vector.tensor_copy(out=xT[:, :rows], in_=psT[:, :rows])
                ps = psA.tile([P, P], f32, name="psA", tag="psA")
                nc.tensor.matmul(ps[:rows, :], lhsT=xT[:, :rows], rhs=W_t[:],
                                 start=True, stop=True)
                hs = hpool.tile([P, P], f32, name="hs", tag="hs")
                nc.vector.tensor_scalar(
                    out=hs[:rows, :], in0=ps[:rows, :],
                    scalar1=dr_t[:rows, t:t+1], scalar2=None,
                    op0=mybir.AluOpType.mult)
                nc.sync.dma_start(out=htab[r0:r0+rows, :], in_=hs[:rows, :])

            # aggregation per dst block
            for b in range(NBLK):
                ps = psB.tile([P, P], f32, name="psB", tag="psB")
                kbb = int(kb[b])
                for c in range(kbb):
                    col = int(col_off[b]) + c
                    g = gpool.tile([P, P], f32, name="g", tag="g")
                    nc.gpsimd.indirect_dma_start(
                        out=g[:], out_offset=None, in_=htab[:],
                        in_offset=bass.IndirectOffsetOnAxis(
                            ap=srcs_t[:, col:col+1], axis=0))
                    M = mpool.tile([P, P], f32, name="M", tag="M")
                    nc.vector.tensor_scalar(
                        out=M[:], in0=iota_t[:],
                        scalar1=dstl_t[:, col:col+1], scalar2=None,
                        op0=mybir.AluOpType.is_equal)
                    nc.tensor.matmul(ps[:], lhsT=M[:], rhs=g[:],
                                     start=(c == 0), stop=(c == kbb - 1))
                rows = min(128, NPC - b * 128)
                ob = opool.tile([P, P], f32, name="ob", tag="ob")
                nc.vector.tensor_scalar(
                    out=ob[:], in0=ps[:], scalar1=dd_t[:, b:b+1],
                    scalar2=None, op0=mybir.AluOpType.mult)
                nc.vector.tensor_tensor(out=ob[:], in0=ob[:], in1=brep_t[:],
                                        op=mybir.AluOpType.add)
                nc.vector.tensor_scalar(
                    out=ob[:], in0=ob[:], scalar1=thr_t[:, :1], scalar2=None,
                    op0=mybir.AluOpType.max)
                nc.sync.dma_start(out=out[b*128:b*128+rows, :],
                                  in_=ob[:rows, :])
    nc.compile()
    return nc


def _build_decode():
    """Decode: out[p, c] = sum_f z[i[p,c], f] * z[j[p,c], f]."""
    CC = EL // NCORES // 128  # 1024 chunks
    nc = bacc.Bacc("TRN2", target_bir_lowering=False, debug=False,
                   num_devices=NCORES)
    z = nc.dram_tensor("z", [N, P], f32, kind="ExternalInput").ap()
    ii = nc.dram_tensor("ii", [P, CC], i32, kind="ExternalInput").ap()
    jj = nc.dram_tensor("jj", [P, CC], i32, kind="ExternalInput").ap()
    o = nc.dram_tensor("o", [P, CC], f32, kind="ExternalOutput").ap()
    GD = 24
    with tile.TileContext(nc) as tc:
        with tc.tile_pool(name="sbuf", bufs=1) as pool:
            ii_t = pool.tile([P, CC], i32, name="ii_t")
            nc.sync.dma_start(out=ii_t[:], in_=ii[:])
            jj_t = pool.tile([P, CC], i32, name="jj_t")
            nc.sync.dma_start(out=jj_t[:], in_=jj[:])
            oc = pool.tile([P, CC], f32, name="oc")
            for c in range(CC):
                gi_ = pool.tile([P, P], f32, name=f"gi{c%GD}", tag=f"gi{c%GD}")
                nc.gpsimd.indirect_dma_start(
                    out=gi_[:], out_offset=None, in_=z[:],
                    in_offset=bass.IndirectOffsetOnAxis(
                        ap=ii_t[:, c:c+1], axis=0))
                gj_ = pool.tile([P, P], f32, name=f"gj{c%GD}", tag=f"gj{c%GD}")
                nc.gpsimd.indirect_dma_start(
                    out=gj_[:], out_offset=None, in_=z[:],
                    in_offset=bass.IndirectOffsetOnAxis(
                        ap=jj_t[:, c:c+1], axis=0))
                pr = pool.tile([P, P], f32, name=f"pr{c%8}", tag=f"pr{c%8}")
                nc.vector.tensor_tensor(out=pr[:], in0=gi_[:], in1=gj_[:],
                                        op=mybir.AluOpType.mult)
                nc.vector.tensor_reduce(
                    out=oc[:, c:c+1], in_=pr[:], axis=mybir.AxisListType.X,
                    op=mybir.AluOpType.add)
            nc.sync.dma_start(out=o[:], in_=oc[:])
    nc.compile()
    return nc


def _get_programs(meta):
    key = ("progs", meta["ktot"], tuple(meta["kb"].tolist()))
    if key not in _prog_cache:
        _prog_cache[key] = (_build_layer(meta["kb"], meta["col_off"]),
                            _build_decode())
    return _prog_cache[key]


def kernel(x, W1, b1, W2, b2, edge_index, edge_label_idx):
    x = np.asarray(x, dtype=np.float32)
    W1 = np.asarray(W1, dtype=np.float32)
    b1 = np.asarray(b1, dtype=np.float32)
    W2 = np.asarray(W2, dtype=np.float32)
    b2 = np.asarray(b2, dtype=np.float32)
    eidx = np.asarray(edge_index)
    eli = np.asarray(edge_label_idx)

    meta = _prep(eidx)
    nc_layer, nc_dec = _get_programs(meta)

    iota = np.broadcast_to(np.arange(P, dtype=np.float32)[None, :],
                           (P, P)).copy()
    W2p = np.zeros((P, P), np.float32)
    W2p[:, :OUT] = W2
    b1rep = np.broadcast_to(b1[None, :], (P, P)).copy().astype(np.float32)
    b2rep = np.zeros((P, P), np.float32)
    b2rep[:, :OUT] = b2[None, :]
    thr_relu = np.zeros((P, 1), np.float32)
    thr_id = np.full((P, 1), -1e30, np.float32)

    def layer_maps(tblv, Wv, brv, thv):
        return [
            {"tbl": tblv, "W": Wv, "brep": brv, "thr": thv, "iota": iota,
             "srcs": meta["srcs"][c], "dstl": meta["dstl"][c],
             "dinv_dst": meta["dinv_dst"][c], "dinv_rows": meta["dinv_rows"]}
            for c in range(NCORES)
        ]

    core_ids = list(range(NCORES))
    # layer 1
    res1 = run_bass_kernel_spmd(
        nc_layer, layer_maps(x, W1, b1rep, thr_relu), core_ids)
    h1 = np.concatenate([res1.results[c]["out"] for c in range(NCORES)],
                        axis=0)
    # layer 2 (padded to 128 feats; cols 64.. are exactly 0)
    res2 = run_bass_kernel_spmd(
        nc_layer, layer_maps(h1, W2p, b2rep, thr_id), core_ids)
    zfull = np.concatenate([res2.results[c]["out"] for c in range(NCORES)],
                           axis=0)
    # decode
    PPC = EL // NCORES
    CC = PPC // 128
    dec_maps = []
    for c in range(NCORES):
        i0 = np.asarray(eli[0][c*PPC:(c+1)*PPC], dtype=np.int32)
        j0 = np.asarray(eli[1][c*PPC:(c+1)*PPC], dtype=np.int32)
        dec_maps.append({"z": zfull,
                         "ii": i0.reshape(CC, 128).T.copy(),
                         "jj": j0.reshape(CC, 128).T.copy()})
    res3 = run_bass_kernel_spmd(nc_dec, dec_maps, core_ids)
    out = np.concatenate(
        [res3.results[c]["o"].T.reshape(-1) for c in range(NCORES)])
    return out.astype(np.float32)


# revision 7
# speedup vs baseline: 32.3317x; 32.3317x over previous
"""GCN link-prediction kernel for 8 Trainium2 NeuronCores.

Strategy:
  - Nodes (dst) sharded across 8 cores (12500 each); each core processes the
    edges whose dst lands in its shard (plus its self-loops).
  - GCN sym-norm factorizes per node: out[d] = dinv[d] * sum_{s in N(d)+d}
    dinv[s]*h[s].  So the per-edge norm disappears: pre-scale rows by dinv
    when writing the dense-matmul result, post-scale the aggregation by dinv.
  - Aggregation: edges sorted by dst block (128 nodes); per 128-edge chunk:
    indirect-DMA gather of h~[src] rows -> one-hot selection matrix built
    on-device from dst_local via iota/is_equal -> PE matmul accumulated in
    PSUM per dst block.
  - One compiled program serves both GCN layers (W2 zero-padded to 128 cols;
    relu vs identity via per-partition threshold input: max(v, thr)).
  - Decode runs as a second program: gather z[i], z[j] per 128-pair chunk,
    multiply + row-reduce on DVE.
Host does index-only prep (degree, sorting, padding) and inter-program
concatenation of shards.
"""
import numpy as np

import concourse.bass as bass
import concourse.bacc as bacc
import concourse.mybir as mybir
import concourse.tile as tile
from concourse.bass_utils import run_bass_kernel_spmd
from concourse.masks import make_identity

f32 = mybir.dt.float32
i32 = mybir.dt.int32

N = 100000
E = 1600000
EL = 1048576
IN = 128
HID = 128
OUT = 64
NCORES = 8
NPC = N // NCORES          # 12500 nodes per core
NBLK = (NPC + 127) // 128  # 98 dst blocks per core
P = 128

_prog_cache = {}


def _prep(edge_index):
    src = np.asarray(edge_index[0], dtype=np.int64)
    dst = np.asarray(edge_index[1], dtype=np.int64)
    deg = np.bincount(dst, minlength=N).astype(np.float64) + 1.0
    dinv = (1.0 / np.sqrt(deg)).astype(np.float32)

    # per (core, block) edge lists, self-loops included
    core_of = dst // NPC
    per_core = []
    counts = np.zeros((NCORES, NBLK), dtype=np.int64)
    for c in range(NCORES):
        m = core_of == c
        s_c = src[m]
        d_c = dst[m] - c * NPC
        loop = np.arange(NPC, dtype=np.int64)
        s_c = np.concatenate([s_c, loop + c * NPC])
        d_c = np.concatenate([d_c, loop])
        blk = d_c // 128
        order = np.argsort(blk, kind="stable")
        s_c, d_c, blk = s_c[order], d_c[order], blk[order]
        per_core.append((s_c, d_c, blk))
        counts[c] = np.bincount(blk, minlength=NBLK)

    kb = ((counts.max(axis=0) + 127) // 128).astype(np.int64)  # chunks per blk
    ktot = int(kb.sum())
    col_off = np.concatenate([[0], np.cumsum(kb)[:-1]])

    srcs = np.zeros((NCORES, 128, ktot), dtype=np.int32)
    dstl = np.full((NCORES, 128, ktot), 999.0, dtype=np.float32)
    for c in range(NCORES):
        s_c, d_c, blk = per_core[c]
        pos = 0
        for b in range(NBLK):
            m = int(counts[c, b])
            cols = int(kb[b])
            spad = np.zeros(cols * 128, dtype=np.int32)
            dpad = np.full(cols * 128, 999.0, dtype=np.float32)
            spad[:m] = s_c[pos:pos + m]
            dpad[:m] = (d_c[pos:pos + m] % 128).astype(np.float32)
            srcs[c, :, col_off[b]:col_off[b] + cols] = spad.reshape(cols, 128).T
            dstl[c, :, col_off[b]:col_off[b] + cols] = dpad.reshape(cols, 128).T
            pos += m

    # dinv arranged per dst block [128, NBLK] and per row tile [128, NT]
    NT = (N + 127) // 128
    dinv_dst = np.ones((NCORES, 128, NBLK), dtype=np.float32)
    for c in range(NCORES):
        v = dinv[c * NPC:(c + 1) * NPC]
        vp = np.ones(NBLK * 128, dtype=np.float32)
        vp[:NPC] = v
        dinv_dst[c] = vp.reshape(NBLK, 128).T
    dr = np.ones(NT * 128, dtype=np.float32)
    dr[:N] = dinv
    dinv_rows = dr.reshape(NT, 128).T.copy()

    return dict(srcs=srcs, dstl=dstl, kb=kb, col_off=col_off,
                dinv_dst=dinv_dst, dinv_rows=dinv_rows, ktot=ktot)


def _build_layer(kb, col_off):
    """One GCN layer: tbl [N,128] -> shard out [NPC,128]."""
    NT = (N + 127) // 128
    nc = bacc.Bacc("TRN2", target_bir_lowering=False, debug=False,
                   num_devices=NCORES)
    tbl = nc.dram_tensor("tbl", [N, P], f32, kind="ExternalInput").ap()
    W = nc.dram_tensor("W", [P, P], f32, kind="ExternalInput").ap()
    brep = nc.dram_tensor("brep", [P, P], f32, kind="ExternalInput").ap()
    thr = nc.dram_tensor("thr", [P, 1], f32, kind="ExternalInput").ap()
    iota = nc.dram_tensor("iota", [P, P], f32, kind="ExternalInput").ap()
    ktot = int(kb.sum())
    srcs = nc.dram_tensor("srcs", [P, ktot], i32, kind="ExternalInput").ap()
    dstl = nc.dram_tensor("dstl", [P, ktot], f32, kind="ExternalInput").ap()
    dinv_dst = nc.dram_tensor("dinv_dst", [P, NBLK], f32,
                              kind="ExternalInput").ap()
    dinv_rows = nc.dram_tensor("dinv_rows", [P, NT], f32,
                               kind="ExternalInput").ap()
    out = nc.dram_tensor("out", [NPC, P], f32, kind="ExternalOutput").ap()

    with tile.TileContext(nc) as tc:
        with (tc.tile_pool(name="const", bufs=1) as cpool,
              tc.tile_pool(name="xin", bufs=8) as xpool,
              tc.tile_pool(name="hs", bufs=8) as hpool,
              tc.tile_pool(name="g", bufs=32) as gpool,
              tc.tile_pool(name="m", bufs=16) as mpool,
              tc.tile_pool(name="ob", bufs=8) as opool,
              tc.tile_pool(name="dram", bufs=1, space="DRAM") as dpool):
            htab = dpool.tile([N, P], f32, name="htab")
            W_t = cpool.tile([P, P], f32, name="W_t")
            nc.sync.dma_start(out=W_t[:], in_=W[:])
            brep_t = cpool.tile([P, P], f32, name="brep_t")
            nc.sync.dma_start(out=brep_t[:], in_=brep[:])
            thr_t = cpool.tile([P, 1], f32, name="thr_t")
            nc.sync.dma_start(out=thr_t[:], in_=thr[:])
            iota_t = cpool.tile([P, P], f32, name="iota_t")
            nc.sync.dma_start(out=iota_t[:], in_=iota[:])
            srcs_t = cpool.tile([P, ktot], i32, name="srcs_t")
            nc.sync.dma_start(out=srcs_t[:], in_=srcs[:])
            dstl_t = cpool.tile([P, ktot], f32, name="dstl_t")
            nc.sync.dma_start(out=dstl_t[:], in_=dstl[:])
            dd_t = cpool.tile([P, NBLK], f32, name="dd_t")
            nc.sync.dma_start(out=dd_t[:], in_=dinv_dst[:])
            dr_t = cpool.tile([P, NT], f32, name="dr_t")
            nc.sync.dma_start(out=dr_t[:], in_=dinv_rows[:])
            ident = cpool.tile([P, P], f32, name="ident")
            make_identity(nc, ident[:])

            # dense: htab = dinv * (tbl @ W)
            with tc.tile_pool(name="psA", bufs=2, space="PSUM") as psA:
                for t in range(NT):
                    r0 = t * 128
                    rows = min(128, N - r0)
                    xt_ = xpool.tile([P, P], f32, name="xt", tag="xt")
                    nc.sync.dma_start(out=xt_[:rows, :],
                                      in_=tbl[r0:r0+rows, :])
                    psT = psA.tile([P, P], f32, name="psT", tag="psT")
                    nc.tensor.transpose(out=psT[:, :rows], in_=xt_[:rows, :],
                                        identity=ident[:rows, :rows])
                    xT = xpool.tile([P, P], f32, name="xT", tag="xT")
                    nc.vector.tensor_copy(out=xT[:, :rows], in_=psT[:, :rows])
                    ps = psA.tile([P, P], f32, name="psA", tag="psA")
                    nc.tensor.matmul(ps[:rows, :], lhsT=xT[:, :rows],
                                     rhs=W_t[:], start=True, stop=True)
                    hs = hpool.tile([P, P], f32, name="hs", tag="hs")
                    nc.vector.tensor_scalar(
                        out=hs[:rows, :], in0=ps[:rows, :],
                        scalar1=dr_t[:rows, t:t+1], scalar2=None,
                        op0=mybir.AluOpType.mult)
                    nc.sync.dma_start(out=htab[r0:r0+rows, :],
                                      in_=hs[:rows, :])

            # aggregation: interleave G dst blocks round-robin so many
            # independent gather->matmul chains are in flight at once
            G = 8
            with tc.tile_pool(name="psB", bufs=1, space="PSUM") as psB:
                for b0 in range(0, NBLK, G):
                    blocks = list(range(b0, min(b0 + G, NBLK)))
                    pstiles = {}
                    for i, b in enumerate(blocks):
                        pstiles[b] = psB.tile([P, P], f32, name="psB",
                                              tag=f"psB{i}")
                    kmax = max(int(kb[b]) for b in blocks)
                    for k in range(kmax):
                        for b in blocks:
                            kbb = int(kb[b])
                            if k >= kbb:
                                continue
                            col = int(col_off[b]) + k
                            g = gpool.tile([P, P], f32, name="g", tag="g")
                            nc.gpsimd.indirect_dma_start(
                                out=g[:], out_offset=None, in_=htab[:],
                                in_offset=bass.IndirectOffsetOnAxis(
                                    ap=srcs_t[:, col:col+1], axis=0))
                            M = mpool.tile([P, P], f32, name="M", tag="M")
                            nc.vector.tensor_scalar(
                                out=M[:], in0=iota_t[:],
                                scalar1=dstl_t[:, col:col+1], scalar2=None,
                                op0=mybir.AluOpType.is_equal)
                            nc.tensor.matmul(pstiles[b][:], lhsT=M[:],
                                             rhs=g[:], start=(k == 0),
                                             stop=(k == kbb - 1))
                    for b in blocks:
                        rows = min(128, NPC - b * 128)
                        ob = opool.tile([P, P], f32, name="ob", tag="ob")
                        nc.vector.tensor_scalar(
                            out=ob[:], in0=pstiles[b][:],
                            scalar1=dd_t[:, b:b+1], scalar2=None,
                            op0=mybir.AluOpType.mult)
                        nc.vector.tensor_tensor(
                            out=ob[:], in0=ob[:], in1=brep_t[:],
                            op=mybir.AluOpType.add)
                        nc.vector.tensor_scalar(
                            out=ob[:], in0=ob[:], scalar1=thr_t[:, :1],
                            scalar2=None, op0=mybir.AluOpType.max)
                        nc.sync.dma_start(out=out[b*128:b*128+rows, :],
                                          in_=ob[:rows, :])
    nc.compile()
    return nc


def _build_decode():
    """Decode: out[p, c] = sum_f z[i[p,c], f] * z[j[p,c], f]."""
    CC = EL // NCORES // 128  # 1024 chunks
    nc = bacc.Bacc("TRN2", target_bir_lowering=False, debug=False,
                   num_devices=NCORES)
    z = nc.dram_tensor("z", [N, P], f32, kind="ExternalInput").ap()
    ii = nc.dram_tensor("ii", [P, CC], i32, kind="ExternalInput").ap()
    jj = nc.dram_tensor("jj", [P, CC], i32, kind="ExternalInput").ap()
    o = nc.dram_tensor("o", [P, CC], f32, kind="ExternalOutput").ap()
    GD = 24
    with tile.TileContext(nc) as tc:
        with tc.tile_pool(name="sbuf", bufs=1) as pool:
            ii_t = pool.tile([P, CC], i32, name="ii_t")
            nc.sync.dma_start(out=ii_t[:], in_=ii[:])
            jj_t = pool.tile([P, CC], i32, name="jj_t")
            nc.sync.dma_start(out=jj_t[:], in_=jj[:])
            oc = pool.tile([P, CC], f32, name="oc")
            for c in range(CC):
                gi_ = pool.tile([P, P], f32, name=f"gi{c%GD}", tag=f"gi{c%GD}")
                nc.gpsimd.indirect_dma_start(
                    out=gi_[:], out_offset=None, in_=z[:],
                    in_offset=bass.IndirectOffsetOnAxis(
                        ap=ii_t[:, c:c+1], axis=0))
                gj_ = pool.tile([P, P], f32, name=f"gj{c%GD}", tag=f"gj{c%GD}")
                nc.gpsimd.indirect_dma_start(
                    out=gj_[:], out_offset=None, in_=z[:],
                    in_offset=bass.IndirectOffsetOnAxis(
                        ap=jj_t[:, c:c+1], axis=0))
                pr = pool.tile([P, P], f32, name=f"pr{c%8}", tag=f"pr{c%8}")
                nc.vector.tensor_tensor(out=pr[:], in0=gi_[:], in1=gj_[:],
                                        op=mybir.AluOpType.mult)
                nc.vector.tensor_reduce(
                    out=oc[:, c:c+1], in_=pr[:], axis=mybir.AxisListType.X,
                    op=mybir.AluOpType.add)
            nc.sync.dma_start(out=o[:], in_=oc[:])
    nc.compile()
    return nc


def _get_programs(meta):
    key = ("progs", meta["ktot"], tuple(meta["kb"].tolist()))
    if key not in _prog_cache:
        _prog_cache[key] = (_build_layer(meta["kb"], meta["col_off"]),
                            _build_decode())
    return _prog_cache[key]


def kernel(x, W1, b1, W2, b2, edge_index, edge_label_idx):
    x = np.asarray(x, dtype=np.float32)
    W1 = np.asarray(W1, dtype=np.float32)
    b1 = np.asarray(b1, dtype=np.float32)
    W2 = np.asarray(W2, dtype=np.float32)
    b2 = np.asarray(b2, dtype=np.float32)
    eidx = np.asarray(edge_index)
    eli = np.asarray(edge_label_idx)

    meta = _prep(eidx)
    nc_layer, nc_dec = _get_programs(meta)

    iota = np.broadcast_to(np.arange(P, dtype=np.float32)[None, :],
                           (P, P)).copy()
    W2p = np.zeros((P, P), np.float32)
    W2p[:, :OUT] = W2
    b1rep = np.broadcast_to(b1[None, :], (P, P)).copy().astype(np.float32)
    b2rep = np.zeros((P, P), np.float32)
    b2rep[:, :OUT] = b2[None, :]
    thr_relu = np.zeros((P, 1), np.float32)
    thr_id = np.full((P, 1), -1e30, np.float32)

    def layer_maps(tblv, Wv, brv, thv):
        return [
            {"tbl": tblv, "W": Wv, "brep": brv, "thr": thv, "iota": iota,
             "srcs": meta["srcs"][c], "dstl": meta["dstl"][c],
             "dinv_dst": meta["dinv_dst"][c], "dinv_rows": meta["dinv_rows"]}
            for c in range(NCORES)
        ]

    core_ids = list(range(NCORES))
    # layer 1
    res1 = run_bass_kernel_spmd(
        nc_layer, layer_maps(x, W1, b1rep, thr_relu), core_ids)
    h1 = np.concatenate([res1.results[c]["out"] for c in range(NCORES)],
                        axis=0)
    # layer 2 (padded to 128 feats; cols 64.. are exactly 0)
    res2 = run_bass_kernel_spmd(
        nc_layer, layer_maps(h1, W2p, b2rep, thr_id), core_ids)
    zfull = np.concatenate([res2.results[c]["out"] for c in range(NCORES)],
                           axis=0)
    # decode
    PPC = EL // NCORES
    CC = PPC // 128
    dec_maps = []
    for c in range(NCORES):
        i0 = np.asarray(eli[0][c*PPC:(c+1)*PPC], dtype=np.int32)
        j0 = np.asarray(eli[1][c*PPC:(c+1)*PPC], dtype=np.int32)
        dec_maps.append({"z": zfull,
                         "ii": i0.reshape(CC, 128).T.copy(),
                         "jj": j0.reshape(CC, 128).T.copy()})
    res3 = run_bass_kernel_spmd(nc_dec, dec_maps, core_ids)
    out = np.concatenate(
        [res3.results[c]["o"].T.reshape(-1) for c in range(NCORES)])
    return out.astype(np.float32)
